# revision 21
# baseline (speedup 1.0000x reference)
"""Trainium2 Bass kernel for an nn.AttentionBlock (GroupNorm -> qkv 1x1 conv ->
single-head self-attention over 32x32 spatial tokens -> proj 1x1 conv ->
residual add).

Full-input contract: kernel(**inputs) takes the complete B=16 batch and
returns the full [16, 512, 32, 32] output. Internally the batch is sharded
2-samples-per-core over 8 NeuronCores (pure data parallelism, no
collectives); the small channel-dim weights are replicated.

Layout strategy (per sample, all fp32):
  x, h, q, k are kept channel-major [C=512, N=1024] (C on partitions, 4
  tiles of [128, 1024]); v is produced token-major [N, C] (8 tiles of
  [128, 512]).  Scores are computed TRANSPOSED (S^T = k^T q, keys on
  partitions) so that exp(S^T) is directly the moving operand of the
  attention*V matmul -- no on-chip transposes anywhere.  Softmax
  denominators come from a ones-column appended to v^T (row 512 of the AV
  output), are reciprocated and broadcast across partitions with a K=1
  matmul, and folded into the AV PSUM evacuation.  GroupNorm group stats are
  per-channel bn_stats aggregated across the 16-channel groups with tiny
  indicator matmuls on the PE (no cross-partition DVE reductions).
"""

import os
import sys
import threading

sys.path.insert(0, "/opt/trn_rl_repo")

import numpy as np

import concourse.bass as bass
import concourse.tile as tile
from concourse import mybir
from concourse.bass_utils import run_bass_kernel_spmd

# ---------------------------------------------------------------------------
# Workaround for this walrus build: CoreV3 codegen accepts at most ONE sync
# wait per instruction (verified empirically -- 2 waits trip "Too many sync
# wait commands").  The Tile scheduler freely attaches several.  Post-pass:
# hoist all but the last wait of each instruction onto preceding single-wait
# NOPs on the same engine.
# ---------------------------------------------------------------------------


def _split_multi_waits(nc, maxw=1):
    seq = 0
    for f in nc.m.functions:
        for bb in f.blocks:
            new_list = []
            changed = False
            for ins in bb.instructions:
                si = getattr(ins, "sync_info", None)
                waits = list(si.on_wait) if si and si.on_wait else []
                if len(waits) > maxw:
                    changed = True
                    for w in waits[:-maxw]:
                        seq += 1
                        new_list.append(
                            mybir.InstNoOp(
                                name=f"I-wsplit-{seq}",
                                engine=ins.engine,
                                sync_info=mybir.SyncInfo(on_wait=[w], on_update=[]),
                                text_hint="wait_split",
                            )
                        )
                    ins.sync_info = mybir.SyncInfo(
                        on_wait=waits[-maxw:], on_update=list(si.on_update)
                    )
                new_list.append(ins)
            if changed:
                bb.instructions[:] = new_list


def _install_axon_ntff_shim():
    """The agent image's `antenv` stub lacks `axon_hooks`, so trace=True would
    be silently skipped.  Recreate the module and register the ctypes-based
    NTFF hook from trn_agent_boot (best effort; timing-only)."""
    try:
        from antenv.axon_hooks import get_axon_ntff_profile_hook  # noqa: F401
        return
    except ImportError:
        pass
    try:
        import types

        import antenv
        from trn_agent_boot.trn_boot import _ntff_profile_via_ctypes

        mod = types.ModuleType("antenv.axon_hooks")
        state = {}
        mod.set_axon_ntff_profile_hook = lambda h: state.__setitem__("h", h)
        mod.get_axon_ntff_profile_hook = lambda: state.get("h")
        sys.modules["antenv.axon_hooks"] = mod
        antenv.axon_hooks = mod
        hook = _ntff_profile_via_ctypes("/opt/axon/libaxon_pjrt.so")
        if hook is not None:
            mod.set_axon_ntff_profile_hook(hook)
    except Exception:
        pass


_install_axon_ntff_shim()

# ---------------------------------------------------------------------------
# Problem constants (hardcoded -- the harness provides no spec files).
# ---------------------------------------------------------------------------

B, C, H, W = 16, 512, 32, 32
N = H * W              # 1024 tokens per sample
GROUPS = 32
GSIZE = C // GROUPS    # 16 channels per group
EPS = 1e-5
NCORES = 8
SPC = B // NCORES      # samples per core
P = 128                # partitions
CT = C // P            # 4 channel tiles
NT = N // P            # 8 token tiles
NH = N // 512          # 2 free-dim halves of the token axis
SCALE = 1.0 / np.sqrt(C)

F32 = mybir.dt.float32
F32R = mybir.dt.float32r


def _build_program(split_waits=True, mm_f32r=True):
    MMDT = F32R if mm_f32r else F32
    nc = bass.Bass()

    xs = nc.dram_tensor("xs", [SPC, C, N], F32, kind="ExternalInput")
    wq = nc.dram_tensor("wq", [P, CT, C], MMDT, kind="ExternalInput")
    wk = nc.dram_tensor("wk", [P, CT, C], MMDT, kind="ExternalInput")
    wv = nc.dram_tensor("wv", [P, CT, C], MMDT, kind="ExternalInput")
    wp = nc.dram_tensor("wp", [P, CT, C], MMDT, kind="ExternalInput")
    qb = nc.dram_tensor("qb", [P, CT], F32, kind="ExternalInput")
    kb = nc.dram_tensor("kb", [P, CT], F32, kind="ExternalInput")
    vbb = nc.dram_tensor("vbb", [P, C], F32, kind="ExternalInput")
    pb = nc.dram_tensor("pb", [P, CT], F32, kind="ExternalInput")
    gnw = nc.dram_tensor("gnw", [P, CT], F32, kind="ExternalInput")
    gnb = nc.dram_tensor("gnb", [P, CT], F32, kind="ExternalInput")
    ind1 = nc.dram_tensor("ind1", [P, CT, GROUPS], F32, kind="ExternalInput")
    ind2 = nc.dram_tensor("ind2", [GROUPS, C], F32, kind="ExternalInput")
    one_c = nc.dram_tensor("one_c", [P, 1], MMDT, kind="ExternalInput")
    one_r = nc.dram_tensor("one_r", [1, P], MMDT, kind="ExternalInput")
    out = nc.dram_tensor("out", [SPC, C, N], F32, kind="ExternalOutput")

    AF = mybir.ActivationFunctionType
    OP = mybir.AluOpType

    with tile.TileContext(nc) as tc:
        ctx_lp = nc.allow_low_precision(reason="fp32r matmul operand rounding")
        ctx_lp.__enter__()
        with (
            tc.tile_pool(name="wpool", bufs=1) as wpool,
            tc.tile_pool(name="xpool", bufs=4) as xpool,
            tc.tile_pool(name="hpool", bufs=4) as hpool,
            tc.tile_pool(name="qpool", bufs=4) as qpool,
            tc.tile_pool(name="kpool", bufs=4) as kpool,
            tc.tile_pool(name="vpool", bufs=8) as vpool,
            tc.tile_pool(name="epool", bufs=8) as epool,
            tc.tile_pool(name="upool", bufs=4) as upool,
            tc.tile_pool(name="rpool", bufs=1) as rpool,
            tc.tile_pool(name="opool", bufs=4) as opool,
            tc.tile_pool(name="aux", bufs=4) as aux,
            tc.tile_pool(name="pmm", bufs=3, space="PSUM") as pmm,
            tc.tile_pool(name="pdn", bufs=1, space="PSUM") as pdn,
        ):
            # ---- PE warm-up: dummy fp32 matmuls while GroupNorm runs ------
            # (fp32 = 4 cyc/col keeps PE busy ~7us with 4 instructions, so the
            # HAM clock gate is at 8/8 when the real matmul stream starts)
            warm_src = wpool.tile([P, 512], F32, name="warm_src")
            nc.vector.memset(warm_src[:], 0.0)
            warm_ps = pdn.tile([1, 512], F32, tag="dn", name="warm_ps")
            for wi in range(12):
                nc.tensor.matmul(
                    warm_ps[:], warm_src[:, 0:1], warm_src[:],
                    start=(wi == 0), stop=(wi == 11),
                )

            # ---- prefetch sample-0 x first: it heads the critical path ----
            xt0 = []
            for ci in range(CT):
                x_t = xpool.tile([P, N], F32, tag="x", name=f"x_0_{ci}")
                for hh in range(NH):
                    nc.sync.dma_start(
                        x_t[:, hh * 512:(hh + 1) * 512],
                        xs[0, ci * P:(ci + 1) * P, hh * 512:(hh + 1) * 512],
                    )
                xt0.append(x_t)

            # ---- resident weights / constants, in first-use order ---------
            gnw_s = wpool.tile([P, CT], F32, name="gnw_s")
            nc.gpsimd.dma_start(gnw_s[:], gnw[:])
            gnb_s = wpool.tile([P, CT], F32, name="gnb_s")
            nc.gpsimd.dma_start(gnb_s[:], gnb[:])
            ind1_s = wpool.tile([P, CT, GROUPS], F32, name="ind1_s")
            nc.gpsimd.dma_start(ind1_s[:], ind1[:])
            ind2_s = wpool.tile([GROUPS, C], F32, name="ind2_s")
            nc.gpsimd.dma_start(ind2_s[:], ind2[:])
            eps_g = wpool.tile([GROUPS, 1], F32, name="eps_g")
            nc.vector.memset(eps_g[:], EPS)
            ones_col = wpool.tile([P, 1], MMDT, name="ones_col")
            nc.gpsimd.dma_start(ones_col[:], one_c[:])
            ones_k1 = wpool.tile([1, P], MMDT, name="ones_k1")
            nc.gpsimd.dma_start(ones_k1[:], one_r[:])
            qb_s = wpool.tile([P, CT], F32, name="qb_s")
            nc.gpsimd.dma_start(qb_s[:], qb[:])
            kb_s = wpool.tile([P, CT], F32, name="kb_s")
            nc.gpsimd.dma_start(kb_s[:], kb[:])
            vbb_s = wpool.tile([P, C], F32, name="vbb_s")
            nc.gpsimd.dma_start(vbb_s[:], vbb[:])
            pb_s = wpool.tile([P, CT], F32, name="pb_s")
            nc.gpsimd.dma_start(pb_s[:], pb[:])
            wq_s = wpool.tile([P, CT, C], MMDT, name="wq_s")
            nc.gpsimd.dma_start(wq_s[:], wq[:])
            wk_s = wpool.tile([P, CT, C], MMDT, name="wk_s")
            nc.gpsimd.dma_start(wk_s[:], wk[:])
            wv_s = wpool.tile([P, CT, C], MMDT, name="wv_s")
            nc.gpsimd.dma_start(wv_s[:], wv[:])
            wp_s = wpool.tile([P, CT, C], MMDT, name="wp_s")
            nc.gpsimd.dma_start(wp_s[:], wp[:])

            def load_x(s):
                xt = []
                for ci in range(CT):
                    x_t = xpool.tile([P, N], F32, tag="x", name=f"x_{s}_{ci}")
                    for hh in range(NH):
                        nc.sync.dma_start(
                            x_t[:, hh * 512:(hh + 1) * 512],
                            xs[s, ci * P:(ci + 1) * P, hh * 512:(hh + 1) * 512],
                        )
                    xt.append(x_t)
                return xt

            def gn_stats(s, xt):
                """bn_stats -> group aggregation -> per-channel (scale, bias)."""
                st2 = []
                for ci in range(CT):
                    stats6 = aux.tile([P, 2, 6], F32, tag="st6", name=f"st6_{s}_{ci}")
                    nc.vector.bn_stats(stats6[:, 0, :], xt[ci][:, 0:512])
                    nc.vector.bn_stats(stats6[:, 1, :], xt[ci][:, 512:1024])
                    mv = aux.tile([P, 2], F32, tag="mv", name=f"mv_{s}_{ci}")
                    nc.vector.bn_aggr(mv[:], stats6[:])
                    s2 = aux.tile([P, 2], F32, tag="s2", name=f"s2_{s}_{ci}")
                    nc.vector.tensor_copy(s2[:, 0:1], mv[:, 0:1])
                    nc.vector.tensor_tensor(s2[:, 1:2], mv[:, 0:1], mv[:, 0:1], OP.mult)
                    nc.vector.tensor_tensor(s2[:, 1:2], s2[:, 1:2], mv[:, 1:2], OP.add)
                    st2.append(s2)

                ps_g = pmm.tile([GROUPS, 2], F32, tag="mm", name=f"psg_{s}")
                for ci in range(CT):
                    nc.tensor.matmul(
                        ps_g[:], ind1_s[:, ci, :], st2[ci][:],
                        start=(ci == 0), stop=(ci == CT - 1),
                    )
                # garr: col0 = mean_g, col1 = rstd_g (via exp(-0.5 ln(var+eps))
                # -- Ln/Exp live in the same ACT table as Exp/Identity/Copy,
                # unlike Sqrt, so the table never swaps mid-kernel)
                gsb = aux.tile([GROUPS, 2], F32, tag="gsb", name=f"gsb_{s}")
                nc.vector.tensor_copy(gsb[:], ps_g[:])
                garr = aux.tile([GROUPS, 6], F32, tag="garr", name=f"garr_{s}")
                nc.vector.tensor_copy(garr[:, 0:1], gsb[:, 0:1])
                nc.vector.tensor_tensor(garr[:, 2:3], gsb[:, 0:1], gsb[:, 0:1], OP.mult)
                nc.vector.tensor_tensor(garr[:, 3:4], gsb[:, 1:2], garr[:, 2:3], OP.subtract)
                nc.scalar.activation(garr[:, 4:5], garr[:, 3:4], AF.Ln, bias=eps_g[:])
                nc.vector.tensor_scalar_mul(garr[:, 5:6], garr[:, 4:5], -0.5)
                nc.scalar.activation(garr[:, 1:2], garr[:, 5:6], AF.Exp)

                scts = []
                for ci in range(CT):
                    ps_c = pmm.tile([P, 2], F32, tag="mm", name=f"psc_{s}_{ci}")
                    nc.tensor.matmul(
                        ps_c[:], ind2_s[:, ci * P:(ci + 1) * P], garr[:, 0:2],
                        start=True, stop=True,
                    )
                    sct = aux.tile([P, 2], F32, tag="sct", name=f"sct_{s}_{ci}")
                    nc.vector.tensor_tensor(
                        sct[:, 0:1], ps_c[:, 1:2], gnw_s[:, ci:ci + 1], OP.mult)
                    nc.vector.tensor_tensor(
                        sct[:, 1:2], ps_c[:, 0:1], sct[:, 0:1], OP.mult)
                    nc.vector.tensor_tensor(
                        sct[:, 1:2], gnb_s[:, ci:ci + 1], sct[:, 1:2], OP.subtract)
                    scts.append(sct)
                return scts

            def gn_apply(s, xt, scts):
                ht = []
                for ci in range(CT):
                    h_t = hpool.tile([P, N], MMDT, tag="h", name=f"h_{s}_{ci}")
                    nc.scalar.activation(
                        h_t[:], xt[ci][:], AF.Identity,
                        bias=scts[ci][:, 1:2], scale=scts[ci][:, 0:1],
                    )
                    ht.append(h_t)
                return ht

            def qkv(s, ht):
                qt, kt = [], []
                for which, wmat, bias_s, dst in (
                    ("q", wq_s, qb_s, qt),
                    ("k", wk_s, kb_s, kt),
                ):
                    for mi in range(CT):
                        acc = pmm.tile([P, N], F32, tag="mm", name=f"{which}ps_{s}_{mi}")
                        for ki in range(CT):
                            for ni in range(NH):
                                nc.tensor.matmul(
                                    acc[:, ni * 512:(ni + 1) * 512],
                                    wmat[:, ki, mi * P:(mi + 1) * P],
                                    ht[ki][:, ni * 512:(ni + 1) * 512],
                                    start=(ki == 0), stop=(ki == CT - 1),
                                )
                        t = (qpool if which == "q" else kpool).tile(
                            [P, N], MMDT, tag=which, name=f"{which}_{s}_{mi}")
                        nc.vector.tensor_scalar(
                            t[:], acc[:],
                            scalar1=bias_s[:, mi:mi + 1], scalar2=None,
                            op0=OP.add,
                        )
                        dst.append(t)

                vt = []
                for ti in range(NT):
                    acc = pmm.tile([P, 512], F32, tag="mm", name=f"vps_{s}_{ti}")
                    for ki in range(CT):
                        nc.tensor.matmul(
                            acc[:],
                            ht[ki][:, ti * P:(ti + 1) * P],
                            wv_s[:, ki, :],
                            start=(ki == 0), stop=(ki == CT - 1),
                        )
                    v_t = vpool.tile([P, C], MMDT, tag="v", name=f"v_{s}_{ti}")
                    nc.vector.tensor_tensor(v_t[:], acc[:], vbb_s[:], OP.add)
                    vt.append(v_t)
                return qt, kt, vt

            def scores(s, qt, kt):
                et = []
                for mi in range(NT):
                    acc = pmm.tile([P, N], F32, tag="mm", name=f"sps_{s}_{mi}")
                    for ci in range(CT):
                        for ni in range(NH):
                            nc.tensor.matmul(
                                acc[:, ni * 512:(ni + 1) * 512],
                                kt[ci][:, mi * P:(mi + 1) * P],
                                qt[ci][:, ni * 512:(ni + 1) * 512],
                                start=(ci == 0), stop=(ci == CT - 1),
                            )
                    e_t = epool.tile([P, N], MMDT, tag="e", name=f"e_{s}_{mi}")
                    nc.scalar.activation(e_t[:], acc[:], AF.Exp, scale=SCALE)
                    et.append(e_t)
                return et

            def av(s, vt, et):
                dn = pdn.tile([1, N], F32, tag="dn", name=f"dn_{s}")
                for ki in range(NT):
                    for ni in range(NH):
                        nc.tensor.matmul(
                            dn[:, ni * 512:(ni + 1) * 512], ones_col[:],
                            et[ki][:, ni * 512:(ni + 1) * 512],
                            start=(ki == 0), stop=(ki == NT - 1),
                        )
                denom = rpool.tile([1, N], F32, tag="denom", name=f"den_{s}")
                nc.scalar.copy(denom[:], dn[:])
                recip = rpool.tile([1, N], MMDT, tag="recip", name=f"rec_{s}")
                nc.vector.reciprocal(recip[:], denom[:])

                rb = rpool.tile([P, N], F32, tag="rb", name=f"rb_{s}")
                ut = []
                for mi in range(CT):
                    acc = pmm.tile([P, N], F32, tag="mm", name=f"avps_{s}_{mi}")
                    for ki in range(NT):
                        for ni in range(NH):
                            nc.tensor.matmul(
                                acc[:, ni * 512:(ni + 1) * 512],
                                vt[ki][:, mi * P:(mi + 1) * P],
                                et[ki][:, ni * 512:(ni + 1) * 512],
                                start=(ki == 0), stop=(ki == NT - 1),
                            )
                    if mi == 0:
                        ps_rb = pdn.tile([P, N], F32, tag="dn", name=f"rbps_{s}")
                        for ni in range(NH):
                            nc.tensor.matmul(
                                ps_rb[:, ni * 512:(ni + 1) * 512], ones_k1[:],
                                recip[:, ni * 512:(ni + 1) * 512],
                                start=True, stop=True,
                            )
                        nc.vector.tensor_copy(rb[:], ps_rb[:])
                    u_t = upool.tile([P, N], MMDT, tag="u", name=f"u_{s}_{mi}")
                    nc.vector.tensor_tensor(u_t[:], acc[:], rb[:], OP.mult)
                    ut.append(u_t)
                return ut

            def proj(s, ut):
                xr = []
                for ci in range(CT):
                    xr_t = xpool.tile([P, N], F32, tag="x", name=f"xr_{s}_{ci}")
                    nc.sync.dma_start(xr_t[:], xs[s, ci * P:(ci + 1) * P, :])
                    xr.append(xr_t)

                for mi in range(CT):
                    acc = pmm.tile([P, N], F32, tag="mm", name=f"pps_{s}_{mi}")
                    for ki in range(CT):
                        for ni in range(NH):
                            nc.tensor.matmul(
                                acc[:, ni * 512:(ni + 1) * 512],
                                wp_s[:, ki, mi * P:(mi + 1) * P],
                                ut[ki][:, ni * 512:(ni + 1) * 512],
                                start=(ki == 0), stop=(ki == CT - 1),
                            )
                    o_t = opool.tile([P, N], F32, tag="o", name=f"o_{s}_{mi}")
                    nc.scalar.activation(
                        o_t[:], acc[:], AF.Identity, bias=pb_s[:, mi:mi + 1],
                    )
                    nc.vector.tensor_tensor(o_t[:], o_t[:], xr[mi][:], OP.add)
                    nc.sync.dma_start(
                        out[s, mi * P:(mi + 1) * P, :], o_t[:],
                    )

            # ---- schedule: hoist sample-1 GN stats into sample-0 compute ---
            sct0 = gn_stats(0, xt0)
            ht0 = gn_apply(0, xt0, sct0)
            q0, k0, v0 = qkv(0, ht0)
            xt1 = load_x(1)
            sct1 = gn_stats(1, xt1)
            e0 = scores(0, q0, k0)
            ht1 = gn_apply(1, xt1, sct1)
            u0 = av(0, v0, e0)
            q1, k1, v1 = qkv(1, ht1)
            e1 = scores(1, q1, k1)
            proj(0, u0)
            u1 = av(1, v1, e1)
            proj(1, u1)

        ctx_lp.__exit__(None, None, None)
    if split_waits:
        _split_multi_waits(nc)
    return nc


_CACHE_LOCK = threading.Lock()
_NC_CACHE = {}


def _get_program():
    with _CACHE_LOCK:
        if "nc" not in _NC_CACHE:
            _NC_CACHE["nc"] = _build_program()
        return _NC_CACHE["nc"]


def _prep_weights(gn_w, gn_b, qkv_w, qkv_b, proj_w, proj_b):
    def pt(v):  # [C] -> [P, CT] with c = t*P + p
        return np.ascontiguousarray(v.reshape(CT, P).T)

    def wt(m):  # [C_out, C_in] -> lhsT layout [P, CT, C_out]
        return np.ascontiguousarray(m.T.reshape(CT, P, m.shape[0]).transpose(1, 0, 2))

    ind1 = np.zeros((C, GROUPS), np.float32)
    ind1[np.arange(C), np.arange(C) // GSIZE] = 1.0 / GSIZE
    ind2 = np.zeros((GROUPS, C), np.float32)
    ind2[np.arange(C) // GSIZE, np.arange(C)] = 1.0

    return {
        "wq": wt(qkv_w[0:C]),
        "wk": wt(qkv_w[C:2 * C]),
        "wv": wt(qkv_w[2 * C:3 * C]),
        "wp": wt(proj_w),
        "qb": pt(qkv_b[0:C]),
        "kb": pt(qkv_b[C:2 * C]),
        "vbb": np.ascontiguousarray(
            np.broadcast_to(qkv_b[2 * C:3 * C][None, :], (P, C))
        ),
        "pb": pt(proj_b),
        "gnw": pt(gn_w),
        "gnb": pt(gn_b),
        "ind1": np.ascontiguousarray(
            ind1.reshape(CT, P, GROUPS).transpose(1, 0, 2)
        ),
        "ind2": ind2,
        "one_c": np.ones((P, 1), np.float32),
        "one_r": np.ones((1, P), np.float32),
    }


def kernel(x, gn_w, gn_b, qkv_w, qkv_b, proj_w, proj_b):
    x = np.asarray(x, dtype=np.float32)
    weights = _prep_weights(
        np.asarray(gn_w, np.float32), np.asarray(gn_b, np.float32),
        np.asarray(qkv_w, np.float32), np.asarray(qkv_b, np.float32),
        np.asarray(proj_w, np.float32), np.asarray(proj_b, np.float32),
    )

    xr = x.reshape(B, C, N)
    in_maps = []
    for core in range(NCORES):
        m = dict(weights)
        m["xs"] = np.ascontiguousarray(xr[core * SPC:(core + 1) * SPC])
        in_maps.append(m)

    nc = _get_program()
    trace = bool(int(os.environ.get("BASS_KERNEL_TRACE", "0")))
    kwargs = {}
    if trace:
        kwargs["trace"] = True
        kwargs["tmpdir"] = os.environ.get("BASS_KERNEL_TRACE_DIR") or None
    res = run_bass_kernel_spmd(nc, in_maps, core_ids=list(range(NCORES)), **kwargs)
    if trace:
        kernel.last_results = res

    out = np.concatenate([res.results[i]["out"] for i in range(NCORES)], axis=0)
    return out.reshape(B, C, H, W)
